# revision 12
# baseline (speedup 1.0000x reference)
"""Trainium2 Bass kernel for CustomFlashAttention (B=2, S=2048, D=2048, H=16).

Sharding over 8 NeuronCores: core c handles batch b=c//4 and head-group
hg=c%4 (4 heads of 128 dims = feature cols [hg*512,(hg+1)*512)).
Per core: QKV projections for its cols, causal flash attention for its 4
heads, partial output projection; host sums the 4 partials per batch.

All matmuls run as float32r (TF32-rate fp32 on the PE array, full rate
for free dim >= 256). Softmax skips the max-subtraction (scores are
~N(0,1); |s| < ~8 for this problem size so exp is safe in fp32) and
defers the 1/sum normalization to the attention output. Row sums come
from an all-ones stationary matmul over the same exp(S^T) tiles used
for the P@V matmuls, giving the sums replicated across partitions so
the normalization is a plain elementwise multiply.
"""

import os
import numpy as np

import concourse.bacc as bacc
import concourse.mybir as mybir
import concourse.tile as tile
from concourse.bass_utils import run_bass_kernel_spmd

B = 2
S = 2048
D = 2048
H_PER_CORE = 4
DC = 512          # feature cols per core (4 heads * 128)
HD = 128          # head dim
P = 128
TB = 512          # token block
N_TB = S // TB    # 4
N_KT = S // P     # 16 (128-wide k/token tiles)
FP32 = mybir.dt.float32
FP32R = mybir.dt.float32r
NEG = -30000.0

LAST_RESULTS = None  # BassKernelResults from the most recent run (for test.py)


def build_bass(causal: bool):
    nc = bacc.Bacc(None, target_bir_lowering=False, debug=False)

    xT_d = nc.dram_tensor("xT", [D, S], FP32R, kind="ExternalInput")
    wqT_d = nc.dram_tensor("wqT", [D, DC], FP32R, kind="ExternalInput")
    wkT_d = nc.dram_tensor("wkT", [D, DC], FP32R, kind="ExternalInput")
    wvT_d = nc.dram_tensor("wvT", [D, DC], FP32R, kind="ExternalInput")
    woT_d = nc.dram_tensor("woT", [DC, D], FP32R, kind="ExternalInput")
    g_d = nc.dram_tensor("gmask", [P, 896], FP32, kind="ExternalInput")
    out_d = nc.dram_tensor("out", [S, D], FP32, kind="ExternalOutput")

    x_r = xT_d.rearrange("(ko p) t -> p ko t", p=P)     # [128, 16, 2048]
    wq_r = wqT_d.rearrange("(ko p) m -> p ko m", p=P)   # [128, 16, 512]
    wk_r = wkT_d.rearrange("(ko p) m -> p ko m", p=P)
    wv_r = wvT_d.rearrange("(ko p) m -> p ko m", p=P)
    wo_r = woT_d.rearrange("(h p) n -> p h n", p=P)     # [128, 4, 2048]

    with tile.TileContext(nc) as tc:
        with tc.tile_pool(name="persist", bufs=1) as persist:
            # Persistent activations, feature-on-partition for Q/K,
            # token-on-partition for V. All fp32r (matmul operands).
            qt_s = persist.tile([P, H_PER_CORE, S], FP32R)  # QT [d, h, tok]
            kt_s = persist.tile([P, H_PER_CORE, S], FP32R)  # KT [d, h, tok]
            v_s = persist.tile([P, N_KT, DC], FP32R)        # V [tok%128, tok//128, feat]

            # ---- projections: pass A (Q transposed + V natural), then
            # pass B (K transposed) reusing the same pools/psum tags so
            # no pool-boundary barrier lands between the passes.
            with tc.tile_pool(name="wproj", bufs=1) as wpool, \
                 tc.tile_pool(name="xa", bufs=6) as xpool, \
                 tc.tile_pool(name="pspr", bufs=1, space="PSUM") as pspr:
                # per-kt weight tiles, DMA'd interleaved with the first
                # token block's x tiles so the first matmuls start early
                wq_t = []
                wv_t = []
                wk_t = []
                for kt in range(N_KT):
                    wq_t.append(wpool.tile([P, DC], FP32R, tag=f"wq{kt}",
                                           name=f"wq{kt}"))
                    wv_t.append(wpool.tile([P, DC], FP32R, tag=f"wv{kt}",
                                           name=f"wv{kt}"))
                    wk_t.append(wpool.tile([P, DC], FP32R, tag=f"wk{kt}",
                                           name=f"wk{kt}"))

                for tb in range(N_TB):
                    psums = [pspr.tile([P, TB], FP32, tag=f"pp{i}",
                                       name=f"pp{i}_{tb}")
                             for i in range(8)]
                    for kt in range(N_KT):
                        if tb == 0:
                            nc.sync.dma_start(wq_t[kt][:], wq_r[:, kt, :])
                            nc.sync.dma_start(wv_t[kt][:], wv_r[:, kt, :])
                        xt = xpool.tile([P, TB], FP32R, tag="xt",
                                        name=f"xt_{tb}_{kt}")
                        nc.sync.dma_start(
                            xt[:], x_r[:, kt, tb * TB:(tb + 1) * TB])
                        first, last = kt == 0, kt == N_KT - 1
                        for h in range(H_PER_CORE):
                            nc.tensor.matmul(
                                psums[h][:],
                                wq_t[kt][:, h * HD:(h + 1) * HD],
                                xt[:],
                                start=first, stop=last)
                        for tt in range(4):
                            nc.tensor.matmul(
                                psums[4 + tt][:],
                                xt[:, tt * P:(tt + 1) * P],
                                wv_t[kt][:],
                                start=first, stop=last)
                    for h in range(H_PER_CORE):
                        nc.vector.tensor_copy(
                            out=qt_s[:, h, tb * TB:(tb + 1) * TB],
                            in_=psums[h][:])
                    for tt in range(4):
                        nc.vector.tensor_copy(
                            out=v_s[:, tb * 4 + tt, :],
                            in_=psums[4 + tt][:])

                # pass B: K, reusing psum tags pp0-3 and the xt tag
                for tb in range(N_TB):
                    psums = [pspr.tile([P, TB], FP32, tag=f"pp{i}",
                                       name=f"pk{i}_{tb}")
                             for i in range(H_PER_CORE)]
                    for kt in range(N_KT):
                        if tb == 0:
                            nc.sync.dma_start(wk_t[kt][:], wk_r[:, kt, :])
                        xt = xpool.tile([P, TB], FP32R, tag="xt",
                                        name=f"xt2_{tb}_{kt}")
                        nc.sync.dma_start(
                            xt[:], x_r[:, kt, tb * TB:(tb + 1) * TB])
                        first, last = kt == 0, kt == N_KT - 1
                        for h in range(H_PER_CORE):
                            nc.tensor.matmul(
                                psums[h][:],
                                wk_t[kt][:, h * HD:(h + 1) * HD],
                                xt[:],
                                start=first, stop=last)
                    for h in range(H_PER_CORE):
                        nc.vector.tensor_copy(
                            out=kt_s[:, h, tb * TB:(tb + 1) * TB],
                            in_=psums[h][:])

            # ---- attention + output projection ----
            with tc.tile_pool(name="watt", bufs=1) as wapool, \
                 tc.tile_pool(name="pt", bufs=16) as ptpool, \
                 tc.tile_pool(name="ptmp", bufs=2) as ptmp, \
                 tc.tile_pool(name="ot", bufs=5) as otpool, \
                 tc.tile_pool(name="small", bufs=2) as smpool, \
                 tc.tile_pool(name="outsb", bufs=3) as outsb, \
                 tc.tile_pool(name="pss", bufs=2, space="PSUM") as pss, \
                 tc.tile_pool(name="pso", bufs=2, space="PSUM") as pso, \
                 tc.tile_pool(name="pssum", bufs=2, space="PSUM") as pssum, \
                 tc.tile_pool(name="psout", bufs=2, space="PSUM") as psout:

                wo_s = wapool.tile([P, H_PER_CORE, D], FP32R, tag="wo")
                nc.sync.dma_start(wo_s[:], wo_r[:])
                ones_f = wapool.tile([P, P], FP32, tag="ones_f")
                nc.any.memset(ones_f[:], 1.0)
                ones_s = wapool.tile([P, P], FP32R, tag="ones")
                nc.vector.tensor_copy(out=ones_s[:], in_=ones_f[:])
                if causal:
                    g_s = wapool.tile([P, 896], FP32, tag="g")
                    nc.sync.dma_start(g_s[:], g_d[:])

                for qb in range(N_TB):
                    ot_tiles = []
                    for h in range(H_PER_CORE):
                        nkt = 4 * qb + 4 if causal else N_KT
                        # scores^T -> exp -> PT tiles [key 128, q 512]
                        pt_tiles = []
                        for kt in range(nkt):
                            ps_s = pss.tile([P, TB], FP32, tag="s",
                                            name=f"s_{qb}_{h}_{kt}")
                            nc.tensor.matmul(
                                ps_s[:],
                                kt_s[:, h, kt * P:(kt + 1) * P],
                                qt_s[:, h, qb * TB:(qb + 1) * TB],
                                start=True, stop=True)
                            ptile = ptpool.tile([P, TB], FP32R, tag="p",
                                                name=f"p_{qb}_{h}_{kt}")
                            if causal and kt >= 4 * qb:
                                delta = (kt - 4 * qb) * P
                                masked = ptmp.tile([P, TB], FP32, tag="m",
                                                   name=f"m_{qb}_{h}_{kt}")
                                nc.vector.tensor_tensor(
                                    masked[:], ps_s[:],
                                    g_s[:, 384 - delta:896 - delta],
                                    mybir.AluOpType.add)
                                nc.scalar.activation(
                                    ptile[:], masked[:],
                                    mybir.ActivationFunctionType.Exp)
                            else:
                                nc.scalar.activation(
                                    ptile[:], ps_s[:],
                                    mybir.ActivationFunctionType.Exp)
                            pt_tiles.append(ptile)
                        # O^T[d',q] accumulated over key tiles; replicated
                        # row sums from the all-ones stationary matmul.
                        ps_o = pso.tile([P, TB], FP32, tag="o",
                                        name=f"o_{qb}_{h}")
                        ps_n = pssum.tile([P, TB], FP32, tag="n",
                                          name=f"n_{qb}_{h}")
                        for i in range(nkt):
                            first, last = i == 0, i == nkt - 1
                            nc.tensor.matmul(
                                ps_o[:],
                                v_s[:, i, h * HD:(h + 1) * HD],
                                pt_tiles[i][:],
                                start=first, stop=last)
                            nc.tensor.matmul(
                                ps_n[:],
                                ones_s[:],
                                pt_tiles[i][:],
                                start=first, stop=last)
                        recip = smpool.tile([P, TB], FP32, tag="r",
                                            name=f"r_{qb}_{h}")
                        nc.vector.reciprocal_approx_fast(out=recip[:], in_=ps_n[:])
                        ot = otpool.tile([P, TB], FP32R, tag="ot",
                                         name=f"ot_{qb}_{h}")
                        nc.vector.tensor_tensor(
                            ot[:], ps_o[:], recip[:],
                            mybir.AluOpType.mult)
                        ot_tiles.append(ot)

                    # output projection for this 512-token block
                    for tt in range(4):
                        row0 = qb * TB + tt * P
                        for nb in range(4):
                            ps_out = psout.tile([P, TB], FP32, tag="po",
                                                name=f"po_{qb}_{tt}_{nb}")
                            for h in range(H_PER_CORE):
                                nc.tensor.matmul(
                                    ps_out[:],
                                    ot_tiles[h][:, tt * P:(tt + 1) * P],
                                    wo_s[:, h, nb * TB:(nb + 1) * TB],
                                    start=(h == 0), stop=(h == H_PER_CORE - 1))
                            ob = outsb.tile([P, TB], FP32, tag="ob",
                                            name=f"ob_{qb}_{tt}_{nb}")
                            nc.vector.tensor_copy(out=ob[:], in_=ps_out[:])
                            nc.sync.dma_start(
                                out_d[row0:row0 + P, nb * TB:(nb + 1) * TB],
                                ob[:])

    nc.compile()
    return nc


_BASS_CACHE = {}


def kernel(x, w_q, w_k, w_v, w_o, causal):
    global LAST_RESULTS
    x = np.asarray(x, dtype=np.float32)
    w_q = np.asarray(w_q, dtype=np.float32)
    w_k = np.asarray(w_k, dtype=np.float32)
    w_v = np.asarray(w_v, dtype=np.float32)
    w_o = np.asarray(w_o, dtype=np.float32)
    is_causal = bool(int(causal))

    if is_causal not in _BASS_CACHE:
        _BASS_CACHE[is_causal] = build_bass(is_causal)
    nc = _BASS_CACHE[is_causal]

    scale = np.float32(1.0 / np.sqrt(HD))
    g = np.zeros((P, 896), dtype=np.float32)
    ii = np.arange(P)[:, None]
    uu = np.arange(896)[None, :]
    g[uu < ii + 384] = NEG

    xT = [np.ascontiguousarray(x[b].T) for b in range(B)]
    in_maps = []
    for c in range(8):
        b, hg = divmod(c, 4)
        cols = slice(hg * DC, (hg + 1) * DC)
        in_maps.append({
            "xT": xT[b],
            "wqT": np.ascontiguousarray(w_q[cols, :].T * scale),
            "wkT": np.ascontiguousarray(w_k[cols, :].T),
            "wvT": np.ascontiguousarray(w_v[cols, :].T),
            "woT": np.ascontiguousarray(w_o[:, cols].T),
            "gmask": g,
        })

    trace = bool(os.environ.get("KERNEL_TRACE"))
    try:
        res = run_bass_kernel_spmd(nc, in_maps, list(range(8)), trace=trace)
    except Exception:
        if not trace:
            raise
        res = run_bass_kernel_spmd(nc, in_maps, list(range(8)), trace=False)
    LAST_RESULTS = res

    out = np.zeros((B, S, D), dtype=np.float32)
    for c in range(8):
        b = c // 4
        out[b] += res.results[c]["out"]
    return out


# revision 18
# speedup vs baseline: 1.0963x; 1.0963x over previous
"""Trainium2 Bass kernel for CustomFlashAttention (B=2, S=2048, D=2048, H=16).

Sharding over 8 NeuronCores: core c handles batch b=c//4 and head-group
hg=c%4 (4 heads of 128 dims = feature cols [hg*512,(hg+1)*512)).
Per core: QKV projections for its cols, causal flash attention for its 4
heads, partial output projection; host sums the 4 partials per batch.

All matmuls run as float32r (TF32-rate fp32 on the PE array, full rate
for free dim >= 256). Softmax skips the max-subtraction (scores are
~N(0,1); |s| < ~8 for this problem size so exp is safe in fp32) and
defers the 1/sum normalization to the attention output. Row sums come
from an all-ones stationary matmul over the same exp(S^T) tiles used
for the P@V matmuls, giving the sums replicated across partitions so
the normalization is a plain elementwise multiply.
"""

import os
import numpy as np

import concourse.bacc as bacc
import concourse.mybir as mybir
import concourse.tile as tile
from concourse.bass_utils import run_bass_kernel_spmd

B = 2
S = 2048
D = 2048
H_PER_CORE = 4
DC = 512          # feature cols per core (4 heads * 128)
HD = 128          # head dim
P = 128
TB = 512          # token block
N_TB = S // TB    # 4
N_KT = S // P     # 16 (128-wide k/token tiles)
FP32 = mybir.dt.float32
FP32R = mybir.dt.float32r
NEG = -30000.0

LAST_RESULTS = None  # BassKernelResults from the most recent run (for test.py)


def build_bass(causal: bool):
    nc = bacc.Bacc(None, target_bir_lowering=False, debug=False)

    xT_d = nc.dram_tensor("xT", [D, S], FP32R, kind="ExternalInput")
    wqT_d = nc.dram_tensor("wqT", [D, DC], FP32R, kind="ExternalInput")
    wkT_d = nc.dram_tensor("wkT", [D, DC], FP32R, kind="ExternalInput")
    wvT_d = nc.dram_tensor("wvT", [D, DC], FP32R, kind="ExternalInput")
    woT_d = nc.dram_tensor("woT", [DC, D], FP32R, kind="ExternalInput")
    g_d = nc.dram_tensor("gmask", [P, 896], FP32, kind="ExternalInput")
    out_d = nc.dram_tensor("out", [S, D], FP32, kind="ExternalOutput")

    x_r = xT_d.rearrange("(ko p) t -> p ko t", p=P)     # [128, 16, 2048]
    wq_r = wqT_d.rearrange("(ko p) m -> p ko m", p=P)   # [128, 16, 512]
    wk_r = wkT_d.rearrange("(ko p) m -> p ko m", p=P)
    wv_r = wvT_d.rearrange("(ko p) m -> p ko m", p=P)
    wo_r = woT_d.rearrange("(h p) n -> p h n", p=P)     # [128, 4, 2048]

    with tile.TileContext(nc) as tc:
        with tc.tile_pool(name="persist", bufs=1) as persist:
            # Persistent activations, feature-on-partition for Q/K,
            # token-on-partition for V. All fp32r (matmul operands).
            qt_s = persist.tile([P, H_PER_CORE, S], FP32R)  # QT [d, h, tok]
            kt_s = persist.tile([P, H_PER_CORE, S], FP32R)  # KT [d, h, tok]
            v_s = persist.tile([P, N_KT, DC], FP32R)        # V [tok%128, tok//128, feat]

            # small constants, loaded up front so the attention phase
            # never queues behind projection-phase DMA traffic
            ones_f = persist.tile([P, P], FP32, tag="ones_f")
            nc.any.memset(ones_f[:], 1.0)
            ones_s = persist.tile([P, P], FP32R, tag="ones")
            nc.vector.tensor_copy(out=ones_s[:], in_=ones_f[:])
            if causal:
                g_s = persist.tile([P, 896], FP32, tag="g")
                nc.sync.dma_start(g_s[:], g_d[:])

            # ---- projections: pass A (Q transposed + V natural), then
            # pass B (K transposed) reusing the same pools/psum tags so
            # no pool-boundary barrier lands between the passes.
            with tc.tile_pool(name="xa", bufs=10) as xpool, \
                 tc.tile_pool(name="pspr", bufs=1, space="PSUM") as pspr:
                # pass A: Q (transposed) + V (natural). Per-kt weight
                # tiles, DMA'd interleaved with the first token block's
                # x tiles so the first matmuls start early.
                with tc.tile_pool(name="wprojA", bufs=1) as wpoolA:
                    wq_t = []
                    wv_t = []
                    for kt in range(N_KT):
                        wq_t.append(wpoolA.tile([P, DC], FP32R,
                                                tag=f"wq{kt}",
                                                name=f"wq{kt}"))
                        wv_t.append(wpoolA.tile([P, DC], FP32R,
                                                tag=f"wv{kt}",
                                                name=f"wv{kt}"))

                    for tb in range(N_TB):
                        psums = [pspr.tile([P, TB], FP32, tag=f"pp{i}",
                                           name=f"pp{i}_{tb}")
                                 for i in range(8)]
                        for kt in range(N_KT):
                            if tb == 0:
                                nc.sync.dma_start(wq_t[kt][:],
                                                  wq_r[:, kt, :])
                                nc.sync.dma_start(wv_t[kt][:],
                                                  wv_r[:, kt, :])
                            xt = xpool.tile([P, TB], FP32R, tag="xt",
                                            name=f"xt_{tb}_{kt}")
                            nc.sync.dma_start(
                                xt[:], x_r[:, kt, tb * TB:(tb + 1) * TB])
                            first, last = kt == 0, kt == N_KT - 1
                            for h in range(H_PER_CORE):
                                nc.tensor.matmul(
                                    psums[h][:],
                                    wq_t[kt][:, h * HD:(h + 1) * HD],
                                    xt[:],
                                    start=first, stop=last)
                            for tt in range(4):
                                nc.tensor.matmul(
                                    psums[4 + tt][:],
                                    xt[:, tt * P:(tt + 1) * P],
                                    wv_t[kt][:],
                                    start=first, stop=last)
                        for h in range(H_PER_CORE):
                            nc.vector.tensor_copy(
                                out=qt_s[:, h, tb * TB:(tb + 1) * TB],
                                in_=psums[h][:])
                        for tt in range(4):
                            nc.vector.tensor_copy(
                                out=v_s[:, tb * 4 + tt, :],
                                in_=psums[4 + tt][:])

                # pass B: K, reusing psum tags pp0-3 and the xt tag;
                # wk tiles land in the space wq/wv just freed
                with tc.tile_pool(name="wprojB", bufs=1) as wpoolB:
                    wk_t = []
                    for kt in range(N_KT):
                        wk_t.append(wpoolB.tile([P, DC], FP32R,
                                                tag=f"wk{kt}",
                                                name=f"wk{kt}"))
                    for tb in range(N_TB):
                        psums = [pspr.tile([P, TB], FP32, tag=f"pp{i}",
                                           name=f"pk{i}_{tb}")
                                 for i in range(H_PER_CORE)]
                        for kt in range(N_KT):
                            if tb == 0:
                                nc.sync.dma_start(wk_t[kt][:],
                                                  wk_r[:, kt, :])
                            xt = xpool.tile([P, TB], FP32R, tag="xt",
                                            name=f"xt2_{tb}_{kt}")
                            nc.sync.dma_start(
                                xt[:], x_r[:, kt, tb * TB:(tb + 1) * TB])
                            first, last = kt == 0, kt == N_KT - 1
                            for h in range(H_PER_CORE):
                                nc.tensor.matmul(
                                    psums[h][:],
                                    wk_t[kt][:, h * HD:(h + 1) * HD],
                                    xt[:],
                                    start=first, stop=last)
                        for h in range(H_PER_CORE):
                            nc.vector.tensor_copy(
                                out=kt_s[:, h, tb * TB:(tb + 1) * TB],
                                in_=psums[h][:])

            # ---- attention + output projection ----
            with tc.tile_pool(name="watt", bufs=1) as wapool, \
                 tc.tile_pool(name="pt", bufs=16) as ptpool, \
                 tc.tile_pool(name="ptmp", bufs=2) as ptmp, \
                 tc.tile_pool(name="ot", bufs=5) as otpool, \
                 tc.tile_pool(name="small", bufs=2) as smpool, \
                 tc.tile_pool(name="outsb", bufs=3) as outsb, \
                 tc.tile_pool(name="pss", bufs=2, space="PSUM") as pss, \
                 tc.tile_pool(name="pso", bufs=2, space="PSUM") as pso, \
                 tc.tile_pool(name="pssum", bufs=2, space="PSUM") as pssum, \
                 tc.tile_pool(name="psout", bufs=2, space="PSUM") as psout:

                wo_s = wapool.tile([P, H_PER_CORE, D], FP32R, tag="wo")
                nc.sync.dma_start(wo_s[:], wo_r[:])

                for qb in range(N_TB):
                    ot_tiles = []
                    for h in range(H_PER_CORE):
                        nkt = 4 * qb + 4 if causal else N_KT
                        # scores^T -> exp -> PT tiles [key 128, q 512]
                        # for diagonal key tiles, queries below the tile's
                        # first key are fully masked: compute only columns
                        # [s0:512) (s0 capped at 256 — below that the
                        # narrower matmul is no cheaper in fp32r)
                        pt_tiles = []
                        for kt in range(nkt):
                            diag = causal and kt >= 4 * qb
                            delta = (kt - 4 * qb) * P if diag else 0
                            s0 = min(delta, 256)
                            ps_s = pss.tile([P, TB], FP32, tag="s",
                                            name=f"s_{qb}_{h}_{kt}")
                            nc.tensor.matmul(
                                ps_s[:, s0:],
                                kt_s[:, h, kt * P:(kt + 1) * P],
                                qt_s[:, h, qb * TB + s0:(qb + 1) * TB],
                                start=True, stop=True)
                            ptile = ptpool.tile([P, TB], FP32R, tag="p",
                                                name=f"p_{qb}_{h}_{kt}")
                            if diag:
                                masked = ptmp.tile([P, TB], FP32, tag="m",
                                                   name=f"m_{qb}_{h}_{kt}")
                                nc.vector.tensor_tensor(
                                    masked[:, s0:], ps_s[:, s0:],
                                    g_s[:, 384 - delta + s0:896 - delta],
                                    mybir.AluOpType.add)
                                nc.scalar.activation(
                                    ptile[:, s0:], masked[:, s0:],
                                    mybir.ActivationFunctionType.Exp)
                            else:
                                nc.scalar.activation(
                                    ptile[:], ps_s[:],
                                    mybir.ActivationFunctionType.Exp)
                            pt_tiles.append((ptile, s0))
                        # O^T[d',q] accumulated over key tiles; replicated
                        # row sums from the all-ones stationary matmul.
                        ps_o = pso.tile([P, TB], FP32, tag="o",
                                        name=f"o_{qb}_{h}")
                        ps_n = pssum.tile([P, TB], FP32, tag="n",
                                          name=f"n_{qb}_{h}")
                        for i in range(nkt):
                            first, last = i == 0, i == nkt - 1
                            ptile, s0 = pt_tiles[i]
                            nc.tensor.matmul(
                                ps_o[:, s0:],
                                v_s[:, i, h * HD:(h + 1) * HD],
                                ptile[:, s0:],
                                start=first, stop=last)
                            nc.tensor.matmul(
                                ps_n[:, s0:],
                                ones_s[:],
                                ptile[:, s0:],
                                start=first, stop=last)
                        recip = smpool.tile([P, TB], FP32, tag="r",
                                            name=f"r_{qb}_{h}")
                        nc.vector.reciprocal_approx_fast(out=recip[:], in_=ps_n[:])
                        ot = otpool.tile([P, TB], FP32R, tag="ot",
                                         name=f"ot_{qb}_{h}")
                        nc.vector.tensor_tensor(
                            ot[:], ps_o[:], recip[:],
                            mybir.AluOpType.mult)
                        ot_tiles.append(ot)

                    # output projection for this 512-token block
                    for tt in range(4):
                        row0 = qb * TB + tt * P
                        for nb in range(4):
                            ps_out = psout.tile([P, TB], FP32, tag="po",
                                                name=f"po_{qb}_{tt}_{nb}")
                            for h in range(H_PER_CORE):
                                nc.tensor.matmul(
                                    ps_out[:],
                                    ot_tiles[h][:, tt * P:(tt + 1) * P],
                                    wo_s[:, h, nb * TB:(nb + 1) * TB],
                                    start=(h == 0), stop=(h == H_PER_CORE - 1))
                            ob = outsb.tile([P, TB], FP32, tag="ob",
                                            name=f"ob_{qb}_{tt}_{nb}")
                            nc.vector.tensor_copy(out=ob[:], in_=ps_out[:])
                            nc.sync.dma_start(
                                out_d[row0:row0 + P, nb * TB:(nb + 1) * TB],
                                ob[:])

    nc.compile()
    return nc


_BASS_CACHE = {}


def kernel(x, w_q, w_k, w_v, w_o, causal):
    global LAST_RESULTS
    x = np.asarray(x, dtype=np.float32)
    w_q = np.asarray(w_q, dtype=np.float32)
    w_k = np.asarray(w_k, dtype=np.float32)
    w_v = np.asarray(w_v, dtype=np.float32)
    w_o = np.asarray(w_o, dtype=np.float32)
    is_causal = bool(int(causal))

    if is_causal not in _BASS_CACHE:
        _BASS_CACHE[is_causal] = build_bass(is_causal)
    nc = _BASS_CACHE[is_causal]

    scale = np.float32(1.0 / np.sqrt(HD))
    g = np.zeros((P, 896), dtype=np.float32)
    ii = np.arange(P)[:, None]
    uu = np.arange(896)[None, :]
    g[uu < ii + 384] = NEG

    xT = [np.ascontiguousarray(x[b].T) for b in range(B)]
    in_maps = []
    for c in range(8):
        b, hg = divmod(c, 4)
        cols = slice(hg * DC, (hg + 1) * DC)
        in_maps.append({
            "xT": xT[b],
            "wqT": np.ascontiguousarray(w_q[cols, :].T * scale),
            "wkT": np.ascontiguousarray(w_k[cols, :].T),
            "wvT": np.ascontiguousarray(w_v[cols, :].T),
            "woT": np.ascontiguousarray(w_o[:, cols].T),
            "gmask": g,
        })

    trace = bool(os.environ.get("KERNEL_TRACE"))
    try:
        res = run_bass_kernel_spmd(nc, in_maps, list(range(8)), trace=trace)
    except Exception:
        if not trace:
            raise
        res = run_bass_kernel_spmd(nc, in_maps, list(range(8)), trace=False)
    LAST_RESULTS = res

    out = np.zeros((B, S, D), dtype=np.float32)
    for c in range(8):
        b = c // 4
        out[b] += res.results[c]["out"]
    return out


# revision 21
# speedup vs baseline: 1.0987x; 1.0021x over previous
"""Trainium2 Bass kernel for CustomFlashAttention (B=2, S=2048, D=2048, H=16).

Sharding over 8 NeuronCores: core c handles batch b=c//4 and head-group
hg=c%4 (4 heads of 128 dims = feature cols [hg*512,(hg+1)*512)).
Per core: QKV projections for its cols, causal flash attention for its 4
heads, partial output projection; host sums the 4 partials per batch.

All matmuls run as float32r (TF32-rate fp32 on the PE array, full rate
for free dim >= 256). Softmax skips the max-subtraction (scores are
~N(0,1); |s| < ~8 for this problem size so exp is safe in fp32) and
defers the 1/sum normalization to the attention output. Row sums come
from an all-ones stationary matmul over the same exp(S^T) tiles used
for the P@V matmuls, giving the sums replicated across partitions so
the normalization is a plain elementwise multiply.
"""

import os
import numpy as np

import concourse.bacc as bacc
import concourse.mybir as mybir
import concourse.tile as tile
from concourse.bass_utils import run_bass_kernel_spmd

B = 2
S = 2048
D = 2048
H_PER_CORE = 4
DC = 512          # feature cols per core (4 heads * 128)
HD = 128          # head dim
P = 128
TB = 512          # token block
N_TB = S // TB    # 4
N_KT = S // P     # 16 (128-wide k/token tiles)
FP32 = mybir.dt.float32
FP32R = mybir.dt.float32r
NEG = -30000.0

LAST_RESULTS = None  # BassKernelResults from the most recent run (for test.py)


def build_bass(causal: bool):
    nc = bacc.Bacc(None, target_bir_lowering=False, debug=False)

    xT_d = nc.dram_tensor("xT", [D, S], FP32R, kind="ExternalInput")
    wqT_d = nc.dram_tensor("wqT", [D, DC], FP32R, kind="ExternalInput")
    wkT_d = nc.dram_tensor("wkT", [D, DC], FP32R, kind="ExternalInput")
    wvT_d = nc.dram_tensor("wvT", [D, DC], FP32R, kind="ExternalInput")
    woT_d = nc.dram_tensor("woT", [DC, D], FP32R, kind="ExternalInput")
    g_d = nc.dram_tensor("gmask", [P, 896], FP32, kind="ExternalInput")
    out_d = nc.dram_tensor("out", [S, D], FP32, kind="ExternalOutput")

    x_r = xT_d.rearrange("(ko p) t -> p ko t", p=P)     # [128, 16, 2048]
    wq_r = wqT_d.rearrange("(ko p) m -> p ko m", p=P)   # [128, 16, 512]
    wk_r = wkT_d.rearrange("(ko p) m -> p ko m", p=P)
    wv_r = wvT_d.rearrange("(ko p) m -> p ko m", p=P)
    wo_r = woT_d.rearrange("(h p) n -> p h n", p=P)     # [128, 4, 2048]

    with tile.TileContext(nc) as tc:
        with tc.tile_pool(name="persist", bufs=1) as persist:
            # Persistent activations, feature-on-partition for Q/K,
            # token-on-partition for V. All fp32r (matmul operands).
            qt_s = persist.tile([P, H_PER_CORE, S], FP32R)  # QT [d, h, tok]
            kt_s = persist.tile([P, H_PER_CORE, S], FP32R)  # KT [d, h, tok]
            v_s = persist.tile([P, N_KT, DC], FP32R)        # V [tok%128, tok//128, feat]

            # small constants, loaded up front so the attention phase
            # never queues behind projection-phase DMA traffic
            ones_f = persist.tile([P, P], FP32, tag="ones_f")
            nc.any.memset(ones_f[:], 1.0)
            ones_s = persist.tile([P, P], FP32R, tag="ones")
            nc.vector.tensor_copy(out=ones_s[:], in_=ones_f[:])
            if causal:
                g_s = persist.tile([P, 896], FP32, tag="g")
                nc.sync.dma_start(g_s[:], g_d[:])

            # ---- projections: pass A (Q transposed + V natural), then
            # pass B (K transposed) reusing the same pools/psum tags so
            # no pool-boundary barrier lands between the passes.
            with tc.tile_pool(name="xa", bufs=10) as xpool, \
                 tc.tile_pool(name="pspr", bufs=1, space="PSUM") as pspr:
                # pass A: Q (transposed) + V (natural). Per-kt weight
                # tiles, DMA'd interleaved with the first token block's
                # x tiles so the first matmuls start early.
                with tc.tile_pool(name="wprojA", bufs=1) as wpoolA:
                    wq_t = []
                    wv_t = []
                    for kt in range(N_KT):
                        wq_t.append(wpoolA.tile([P, DC], FP32R,
                                                tag=f"wq{kt}",
                                                name=f"wq{kt}"))
                        wv_t.append(wpoolA.tile([P, DC], FP32R,
                                                tag=f"wv{kt}",
                                                name=f"wv{kt}"))

                    for tb in range(N_TB):
                        psums = [pspr.tile([P, TB], FP32, tag=f"pp{i}",
                                           name=f"pp{i}_{tb}")
                                 for i in range(8)]
                        for kt in range(N_KT):
                            if tb == 0:
                                nc.sync.dma_start(wq_t[kt][:],
                                                  wq_r[:, kt, :])
                                nc.sync.dma_start(wv_t[kt][:],
                                                  wv_r[:, kt, :])
                            xt = xpool.tile([P, TB], FP32R, tag="xt",
                                            name=f"xt_{tb}_{kt}")
                            nc.sync.dma_start(
                                xt[:], x_r[:, kt, tb * TB:(tb + 1) * TB])
                            first, last = kt == 0, kt == N_KT - 1
                            for h in range(H_PER_CORE):
                                nc.tensor.matmul(
                                    psums[h][:],
                                    wq_t[kt][:, h * HD:(h + 1) * HD],
                                    xt[:],
                                    start=first, stop=last)
                            for tt in range(4):
                                nc.tensor.matmul(
                                    psums[4 + tt][:],
                                    xt[:, tt * P:(tt + 1) * P],
                                    wv_t[kt][:],
                                    start=first, stop=last)
                        # split evicts across DVE and the idle Scalar
                        # engine so the boundary chain halves
                        for h in range(H_PER_CORE):
                            nc.vector.tensor_copy(
                                out=qt_s[:, h, tb * TB:(tb + 1) * TB],
                                in_=psums[h][:])
                        for tt in range(4):
                            nc.scalar.copy(
                                out=v_s[:, tb * 4 + tt, :],
                                in_=psums[4 + tt][:])

                # pass B: K, reusing psum tags pp0-3 and the xt tag;
                # wk tiles land in the space wq/wv just freed
                with tc.tile_pool(name="wprojB", bufs=1) as wpoolB:
                    wk_t = []
                    for kt in range(N_KT):
                        wk_t.append(wpoolB.tile([P, DC], FP32R,
                                                tag=f"wk{kt}",
                                                name=f"wk{kt}"))
                    for tb in range(N_TB):
                        psums = [pspr.tile([P, TB], FP32, tag=f"pp{i}",
                                           name=f"pk{i}_{tb}")
                                 for i in range(H_PER_CORE)]
                        for kt in range(N_KT):
                            if tb == 0:
                                nc.sync.dma_start(wk_t[kt][:],
                                                  wk_r[:, kt, :])
                            xt = xpool.tile([P, TB], FP32R, tag="xt",
                                            name=f"xt2_{tb}_{kt}")
                            nc.sync.dma_start(
                                xt[:], x_r[:, kt, tb * TB:(tb + 1) * TB])
                            first, last = kt == 0, kt == N_KT - 1
                            for h in range(H_PER_CORE):
                                nc.tensor.matmul(
                                    psums[h][:],
                                    wk_t[kt][:, h * HD:(h + 1) * HD],
                                    xt[:],
                                    start=first, stop=last)
                        for h in range(H_PER_CORE):
                            if h % 2 == 0:
                                nc.vector.tensor_copy(
                                    out=kt_s[:, h, tb * TB:(tb + 1) * TB],
                                    in_=psums[h][:])
                            else:
                                nc.scalar.copy(
                                    out=kt_s[:, h, tb * TB:(tb + 1) * TB],
                                    in_=psums[h][:])

            # ---- attention + output projection ----
            with tc.tile_pool(name="watt", bufs=1) as wapool, \
                 tc.tile_pool(name="pt", bufs=16) as ptpool, \
                 tc.tile_pool(name="ptmp", bufs=2) as ptmp, \
                 tc.tile_pool(name="ot", bufs=5) as otpool, \
                 tc.tile_pool(name="small", bufs=2) as smpool, \
                 tc.tile_pool(name="outsb", bufs=3) as outsb, \
                 tc.tile_pool(name="pss", bufs=2, space="PSUM") as pss, \
                 tc.tile_pool(name="pso", bufs=2, space="PSUM") as pso, \
                 tc.tile_pool(name="pssum", bufs=2, space="PSUM") as pssum, \
                 tc.tile_pool(name="psout", bufs=2, space="PSUM") as psout:

                wo_s = wapool.tile([P, H_PER_CORE, D], FP32R, tag="wo")
                nc.sync.dma_start(wo_s[:], wo_r[:])

                for qb in range(N_TB):
                    ot_tiles = []
                    for h in range(H_PER_CORE):
                        nkt = 4 * qb + 4 if causal else N_KT
                        # scores^T -> exp -> PT tiles [key 128, q 512]
                        # for diagonal key tiles, queries below the tile's
                        # first key are fully masked: compute only columns
                        # [s0:512) (s0 capped at 256 — below that the
                        # narrower matmul is no cheaper in fp32r)
                        pt_tiles = []
                        for kt in range(nkt):
                            diag = causal and kt >= 4 * qb
                            delta = (kt - 4 * qb) * P if diag else 0
                            s0 = min(delta, 256)
                            ps_s = pss.tile([P, TB], FP32, tag="s",
                                            name=f"s_{qb}_{h}_{kt}")
                            nc.tensor.matmul(
                                ps_s[:, s0:],
                                kt_s[:, h, kt * P:(kt + 1) * P],
                                qt_s[:, h, qb * TB + s0:(qb + 1) * TB],
                                start=True, stop=True)
                            ptile = ptpool.tile([P, TB], FP32R, tag="p",
                                                name=f"p_{qb}_{h}_{kt}")
                            if diag:
                                masked = ptmp.tile([P, TB], FP32, tag="m",
                                                   name=f"m_{qb}_{h}_{kt}")
                                nc.vector.tensor_tensor(
                                    masked[:, s0:], ps_s[:, s0:],
                                    g_s[:, 384 - delta + s0:896 - delta],
                                    mybir.AluOpType.add)
                                nc.scalar.activation(
                                    ptile[:, s0:], masked[:, s0:],
                                    mybir.ActivationFunctionType.Exp)
                            else:
                                nc.scalar.activation(
                                    ptile[:], ps_s[:],
                                    mybir.ActivationFunctionType.Exp)
                            pt_tiles.append((ptile, s0))
                        # O^T[d',q] accumulated over key tiles; replicated
                        # row sums from the all-ones stationary matmul.
                        ps_o = pso.tile([P, TB], FP32, tag="o",
                                        name=f"o_{qb}_{h}")
                        ps_n = pssum.tile([P, TB], FP32, tag="n",
                                          name=f"n_{qb}_{h}")
                        for i in range(nkt):
                            first, last = i == 0, i == nkt - 1
                            ptile, s0 = pt_tiles[i]
                            nc.tensor.matmul(
                                ps_o[:, s0:],
                                v_s[:, i, h * HD:(h + 1) * HD],
                                ptile[:, s0:],
                                start=first, stop=last)
                            nc.tensor.matmul(
                                ps_n[:, s0:],
                                ones_s[:],
                                ptile[:, s0:],
                                start=first, stop=last)
                        recip = smpool.tile([P, TB], FP32, tag="r",
                                            name=f"r_{qb}_{h}")
                        nc.vector.reciprocal_approx_fast(out=recip[:], in_=ps_n[:])
                        ot = otpool.tile([P, TB], FP32R, tag="ot",
                                         name=f"ot_{qb}_{h}")
                        nc.vector.tensor_tensor(
                            ot[:], ps_o[:], recip[:],
                            mybir.AluOpType.mult)
                        ot_tiles.append(ot)

                    # output projection for this 512-token block
                    for tt in range(4):
                        row0 = qb * TB + tt * P
                        for nb in range(4):
                            ps_out = psout.tile([P, TB], FP32, tag="po",
                                                name=f"po_{qb}_{tt}_{nb}")
                            for h in range(H_PER_CORE):
                                nc.tensor.matmul(
                                    ps_out[:],
                                    ot_tiles[h][:, tt * P:(tt + 1) * P],
                                    wo_s[:, h, nb * TB:(nb + 1) * TB],
                                    start=(h == 0), stop=(h == H_PER_CORE - 1))
                            ob = outsb.tile([P, TB], FP32, tag="ob",
                                            name=f"ob_{qb}_{tt}_{nb}")
                            nc.vector.tensor_copy(out=ob[:], in_=ps_out[:])
                            nc.sync.dma_start(
                                out_d[row0:row0 + P, nb * TB:(nb + 1) * TB],
                                ob[:])

    nc.compile()
    return nc


_BASS_CACHE = {}


def kernel(x, w_q, w_k, w_v, w_o, causal):
    global LAST_RESULTS
    x = np.asarray(x, dtype=np.float32)
    w_q = np.asarray(w_q, dtype=np.float32)
    w_k = np.asarray(w_k, dtype=np.float32)
    w_v = np.asarray(w_v, dtype=np.float32)
    w_o = np.asarray(w_o, dtype=np.float32)
    is_causal = bool(int(causal))

    if is_causal not in _BASS_CACHE:
        _BASS_CACHE[is_causal] = build_bass(is_causal)
    nc = _BASS_CACHE[is_causal]

    scale = np.float32(1.0 / np.sqrt(HD))
    g = np.zeros((P, 896), dtype=np.float32)
    ii = np.arange(P)[:, None]
    uu = np.arange(896)[None, :]
    g[uu < ii + 384] = NEG

    xT = [np.ascontiguousarray(x[b].T) for b in range(B)]
    in_maps = []
    for c in range(8):
        b, hg = divmod(c, 4)
        cols = slice(hg * DC, (hg + 1) * DC)
        in_maps.append({
            "xT": xT[b],
            "wqT": np.ascontiguousarray(w_q[cols, :].T * scale),
            "wkT": np.ascontiguousarray(w_k[cols, :].T),
            "wvT": np.ascontiguousarray(w_v[cols, :].T),
            "woT": np.ascontiguousarray(w_o[:, cols].T),
            "gmask": g,
        })

    trace = bool(os.environ.get("KERNEL_TRACE"))
    try:
        res = run_bass_kernel_spmd(nc, in_maps, list(range(8)), trace=trace)
    except Exception:
        if not trace:
            raise
        res = run_bass_kernel_spmd(nc, in_maps, list(range(8)), trace=False)
    LAST_RESULTS = res

    out = np.zeros((B, S, D), dtype=np.float32)
    for c in range(8):
        b = c // 4
        out[b] += res.results[c]["out"]
    return out
